# revision 1
# baseline (speedup 1.0000x reference)
"""Bidirectional LSTM (S=2048, B=4096, I=1, H=8, O=1) on 8 Trainium2 NeuronCores.

Strategy
--------
Pure data parallel over batch (512 rows/core) plus *sequence chunking with
warmup* inside each core: the LSTM forget gate contracts state influence by
~0.5/step, so a chunk that starts W=20 steps early from zero state converges
to the exact trajectory (~3e-5 abs err on h, below the tf32 matmul noise)
before its first emitted output.

Per core the sequence is split into G=2 pipelined groups x NP=7 chunk-streams
per direction (chunk length L=147, zero-padded tail).  The 7 (fwd,bwd) stream
pairs of a group are stacked block-diagonally in the contraction dim together
with their x inputs and a constant ones row:
rhs = [h (7x16=112) ; x (14) ; ones (1)] = [127, 512], so ONE matmul per gate
type computes W_hh.h + w_ih.x + bias for all 7 streams at full PE packing.
Matmuls run in float32r (tf32).  Per round (one step of all streams of a
group):

  PE : 4 gate matmuls (K=127, M=112) + 1 out-proj (M=8, dst partition 0)
  ACT: sigmoid(f,i merged), tanh(g), sigmoid(o), tanh(c)
  DVE: t=f*c, z=i*g, c'=t+z, h'=o*tanh(c'), out-proj psum->sbuf copy
  DMA: next round's x rows into the next rhs tile; out rows to HBM
  (b_out is added host-side after the gather)

Chunk 0 starts exactly at t=0 and must begin with true zero state: during its
W warmup rounds the group-0 matmuls use weight copies whose pair-0 gate
columns are zeroed, which keeps that pair's (h,c) identically 0
(sigmoid(0)=.5, tanh(0)=0 => c'=.5*0+.5*0, h'=.5*tanh(0)=0).

float32r ISA rules: matmul dst start_partition must be 0 (hence out-proj M=8
at partition 0) and operands must be produced as float32r (host pre-rounds
to tf32; on-chip h is written as float32r by the DVE).

PSUM budget (8 banks): sigmoid(f,i) tiles 2x2 + g/o tiles 2x1 + out 2x1.

Measured: ~1.26 ms HW exec across 8 cores, rel err 4.3e-4 (USE_F32R=True);
~1.59 ms at rel err 2.4e-6 with USE_F32R=False (exact fp32 matmuls).
"""

import os
import sys

if "axon" not in os.environ.get("JAX_PLATFORMS", "axon"):
    os.environ["JAX_PLATFORMS"] = "axon,cpu"

try:
    import concourse  # noqa: F401
except ImportError:  # pragma: no cover
    sys.path.insert(0, "/opt/trn_rl_repo")

from contextlib import ExitStack

import numpy as np

import concourse.bacc as bacc
import concourse.mybir as mybir
import concourse.tile as tile

S, B, I, H, O = 2048, 4096, 1, 8, 1
N_CORES = 8
BC = B // N_CORES  # batch columns per core

# sequence-chunking parameters
NP = 7      # stream pairs per group; K = 16*NP + 2*NP + 1 = 127
G = 2       # pipelined groups per core
W = 20      # warmup rounds per chunk
FLUSH = 1   # rounds per out-psum flush window

USE_F32R = True

GATES = ("g", "i", "f", "o")
TORCH_BLOCK = {"i": 0, "f": 1, "g": 2, "o": 3}  # torch LSTM gate row blocks

F32 = mybir.dt.float32
AF = mybir.ActivationFunctionType


def _lchunk(s_len, n_pairs, n_groups):
    n_chunks = n_pairs * n_groups
    return -(-s_len // n_chunks)  # ceil; tail chunk padded with zero x


# --------------------------------------------------------------------------
# host-side data preparation
# --------------------------------------------------------------------------

def tf32_round(a):
    """Round fp32 to tfloat32 (10-bit mantissa) — what fp32r matmuls consume."""
    u = np.ascontiguousarray(a, np.float32).view(np.uint32).copy()
    u = (u + np.uint32(0x1000)) & np.uint32(0xFFFFE000)
    return u.view(np.float32)


def make_weights(wihs, whhs, bihs, bhhs, w_out, n_pairs):
    """Combined block-diagonal stationary operands [K_rhs, 16*n_pairs].

    lhsT rows: 0..16P h-rows, 16P..18P x-rows, last row = bias (ones row).
    wihs/whhs/bihs/bhhs: per-direction lists [fwd, bwd].
    """
    KH = 16 * n_pairs
    KR = KH + 2 * n_pairs + 1
    out = {}
    for t in GATES:
        bi = TORCH_BLOCK[t]
        w = np.zeros((KR, KH), np.float32)
        for s in range(n_pairs):
            for d in range(2):
                c0 = 16 * s + 8 * d
                w[c0:c0 + 8, c0:c0 + 8] = whhs[d][8 * bi:8 * bi + 8, :].T
                w[KH + 2 * s + d, c0:c0 + 8] = wihs[d][8 * bi:8 * bi + 8, 0]
                w[KR - 1, c0:c0 + 8] = (bihs[d] + bhhs[d])[8 * bi:8 * bi + 8]
        w_warm = w.copy(); w_warm[:, 0:16] = 0.0
        out[f"w_{t}"] = w
        out[f"w_{t}_warm"] = w_warm
    wo = np.zeros((KH, 8), np.float32)
    for s in range(n_pairs):
        for d in range(2):
            wo[16 * s + 8 * d:16 * s + 8 * d + 8, s] = w_out[0, 8 * d:8 * d + 8]
    out["w_out"] = wo
    return out


def make_xarr(x_core, future, n_pairs, n_groups, l_chunk, w_warm):
    """Per-core x arranged as [G, R, 2*NP+1, BC]; last row is ones (bias)."""
    s_len, bc = x_core.shape
    R = l_chunk + w_warm
    xb = x_core[(future - np.arange(s_len)) % s_len]
    xarr = np.zeros((n_groups, R, 2 * n_pairs + 1, bc), np.float32)
    xarr[:, :, 2 * n_pairs, :] = 1.0
    rr = np.arange(R)
    for g in range(n_groups):
        for s in range(n_pairs):
            pos = (g * n_pairs + s) * l_chunk - w_warm + rr
            valid = (pos >= 0) & (pos < s_len)
            for d, src in enumerate((x_core, xb)):
                xarr[g, valid, 2 * s + d, :] = src[pos[valid]]
    return xarr


def make_in_maps(x, wihs, whhs, bihs, bhhs, w_out, b_out, future,
                 use_f32r=None):
    if use_f32r is None:
        use_f32r = USE_F32R
    shared = make_weights(wihs, whhs, bihs, bhhs, w_out, NP)
    if use_f32r:
        shared = {k: tf32_round(v) for k, v in shared.items()}
    l_chunk = _lchunk(S, NP, G)
    in_maps = []
    for k in range(N_CORES):
        m = dict(shared)
        xa = make_xarr(x[:, k * BC:(k + 1) * BC, 0], future, NP, G, l_chunk, W)
        m["xarr"] = tf32_round(xa) if use_f32r else xa
        in_maps.append(m)
    return in_maps


# --------------------------------------------------------------------------
# program builder
# --------------------------------------------------------------------------

def build_program(n_pairs=NP, n_groups=G, w_warm=W, bc=BC,
                  s_len=S, use_f32r=USE_F32R, num_devices=N_CORES):
    """Build + compile the Bass/Tile program. Returns (nc, input_names)."""
    l_chunk = _lchunk(s_len, n_pairs, n_groups)
    s_pad = l_chunk * n_pairs * n_groups
    KH = 16 * n_pairs            # h rows / gate-psum partitions
    KR = KH + 2 * n_pairs + 1    # rhs rows (h + x + ones)
    R = l_chunk + w_warm

    nc = bacc.Bacc("TRN2", target_bir_lowering=False, debug=False,
                   enable_asserts=False, num_devices=num_devices)

    dram = {}
    host_names = []

    mmdt = mybir.dt.float32r if use_f32r else F32

    def din(name, shape, dt_=F32):
        dram[name] = nc.dram_tensor(name, list(shape), dt_, kind="ExternalInput").ap()
        host_names.append(name)

    for t in GATES:
        din(f"w_{t}", (KR, KH), mmdt)
        din(f"w_{t}_warm", (KR, KH), mmdt)
    din("w_out", (KH, 8), mmdt)
    din("xarr", (n_groups, R, 2 * n_pairs + 1, bc), mmdt)
    out_d = nc.dram_tensor("out", [s_pad, bc], F32, kind="ExternalOutput").ap()
    out_view = out_d.rearrange("(c l) b -> c l b", l=l_chunk)

    with tile.TileContext(nc) as tc, ExitStack() as ctx:
        consts = ctx.enter_context(tc.tile_pool(name="consts", bufs=1))
        hp = ctx.enter_context(tc.tile_pool(name="hp", bufs=4))
        cp = ctx.enter_context(tc.tile_pool(name="cp", bufs=4))
        up = ctx.enter_context(tc.tile_pool(name="up", bufs=4))
        zp = ctx.enter_context(tc.tile_pool(name="zp", bufs=4))
        osb = ctx.enter_context(tc.tile_pool(name="osb", bufs=3))
        gps = ctx.enter_context(tc.tile_pool(name="gps", bufs=2, space="PSUM"))
        gp1 = ctx.enter_context(tc.tile_pool(name="gp1", bufs=2, space="PSUM"))
        ops = ctx.enter_context(tc.tile_pool(name="ops", bufs=2, space="PSUM"))

        ct = {}
        for name, ap in dram.items():
            if name == "xarr":
                continue
            t_ = consts.tile(list(ap.shape), ap.dtype, name=f"c_{name}", tag=f"c_{name}")
            nc.sync.dma_start(out=t_, in_=ap)
            ct[name] = t_

        rhs_cur, c_prev = [], []
        for g in range(n_groups):
            r0t = hp.tile([KR, bc], mmdt, name=f"rhs0_{g}", tag=f"h{g}")
            nc.vector.memset(r0t[0:KH, :].bitcast(F32), 0.0)
            nc.sync.dma_start(out=r0t[KH:KR, :], in_=dram["xarr"][g, 0])
            c0 = cp.tile([KH, bc], F32, name=f"c0_{g}", tag=f"c{g}")
            nc.vector.memset(c0, 0.0)
            rhs_cur.append(r0t)
            c_prev.append(c0)

        out_ps = [None] * n_groups
        for r in range(R):
            for g in range(n_groups):
                warm = "_warm" if (g == 0 and r < w_warm) else ""
                rhs = rhs_cur[g]

                u = {}
                # mm order: f, i (merged sigmoid), g, o; z-path needs g,i;
                # tm-path needs f; o only feeds h at the end
                fi = gps.tile([KH, 2, bc], F32, name=f"fi_{g}_{r}", tag="gfi")
                nc.tensor.matmul(fi[:, 0, :], ct[f"w_f{warm}"], rhs,
                                 start=True, stop=True)
                nc.tensor.matmul(fi[:, 1, :], ct[f"w_i{warm}"], rhs,
                                 start=True, stop=True)
                psg = gp1.tile([KH, bc], F32, name=f"psg_{g}_{r}", tag="ggo")
                nc.tensor.matmul(psg, ct[f"w_g{warm}"], rhs,
                                 start=True, stop=True)
                u_fi = up.tile([KH, 2, bc], F32, name=f"ufi_{g}_{r}", tag=f"ufi{g}")
                nc.scalar.activation(u_fi, fi, AF.Sigmoid)
                u["f"], u["i"] = u_fi[:, 0, :], u_fi[:, 1, :]
                tm = zp.tile([KH, bc], F32, name=f"t_{g}_{r}", tag=f"tm{g}")
                nc.vector.tensor_mul(tm, u["f"], c_prev[g])
                u["g"] = up.tile([KH, bc], F32, name=f"ug_{g}_{r}", tag=f"ug{g}")
                nc.scalar.activation(u["g"], psg, AF.Tanh)
                pso = gp1.tile([KH, bc], F32, name=f"pso_{g}_{r}", tag="ggo")
                nc.tensor.matmul(pso, ct[f"w_o{warm}"], rhs,
                                 start=True, stop=True)
                u["o"] = up.tile([KH, bc], F32, name=f"uo_{g}_{r}", tag=f"uo{g}")
                nc.scalar.activation(u["o"], pso, AF.Sigmoid)

                z = zp.tile([KH, bc], F32, name=f"z_{g}_{r}", tag=f"z{g}")
                nc.vector.tensor_mul(z, u["i"], u["g"])
                cn = cp.tile([KH, bc], F32, name=f"c_{g}_{r}", tag=f"c{g}")
                nc.vector.tensor_add(cn, tm, z)
                tcn = up.tile([KH, bc], F32, name=f"tc_{g}_{r}", tag=f"u_tc{g}")
                nc.scalar.activation(tcn, cn, AF.Tanh)

                rhs_n = hp.tile([KR, bc], mmdt, name=f"rhs_{g}_{r}", tag=f"h{g}")
                if r + 1 < R:
                    nc.sync.dma_start(out=rhs_n[KH:KR, :], in_=dram["xarr"][g, r + 1])
                nc.vector.tensor_mul(rhs_n[0:KH, :], u["o"], tcn)
                rhs_cur[g], c_prev[g] = rhs_n, cn

                if r >= w_warm:
                    ops_t = ops.tile([8, bc], F32, name=f"ops_{g}_{r}", tag="out")
                    nc.tensor.matmul(ops_t, ct["w_out"],
                                     rhs_n[0:KH, :], start=True, stop=True)
                    ob = osb.tile([8, bc], F32, name=f"ob_{g}_{r}", tag=f"ob{g}")
                    nc.vector.tensor_copy(ob, ops_t)  # b_out added host-side
                    pos = r - w_warm
                    nc.sync.dma_start(
                        out=out_view[g * n_pairs:(g + 1) * n_pairs, pos, :],
                        in_=ob[0:n_pairs, :])

    nc.compile()
    return nc, host_names


# --------------------------------------------------------------------------
# runner
# --------------------------------------------------------------------------

_CACHE = {}


def _get_program(use_f32r=None):
    if use_f32r is None:
        use_f32r = USE_F32R
    key = (NP, G, W, BC, S, use_f32r)
    if key not in _CACHE:
        _CACHE[key] = build_program(use_f32r=use_f32r)
    return _CACHE[key]


def kernel(x, w_ih_f, w_hh_f, b_ih_f, b_hh_f, w_ih_b, w_hh_b, b_ih_b, b_hh_b,
           w_out, b_out, future):
    from concourse import bass_utils

    x = np.asarray(x, np.float32)
    wihs = [np.asarray(w_ih_f, np.float32), np.asarray(w_ih_b, np.float32)]
    whhs = [np.asarray(w_hh_f, np.float32), np.asarray(w_hh_b, np.float32)]
    bihs = [np.asarray(b_ih_f, np.float32), np.asarray(b_ih_b, np.float32)]
    bhhs = [np.asarray(b_hh_f, np.float32), np.asarray(b_hh_b, np.float32)]
    w_out = np.asarray(w_out, np.float32)
    b_out = float(np.asarray(b_out).reshape(-1)[0])
    future = int(future)

    nc, names = _get_program()
    in_maps = make_in_maps(x, wihs, whhs, bihs, bhhs, w_out, b_out, future)
    res = bass_utils.run_bass_kernel_spmd(nc, in_maps, core_ids=list(range(N_CORES)))
    out = np.empty((B, S), np.float32)
    for k in range(N_CORES):
        out[k * BC:(k + 1) * BC, :] = res.results[k]["out"][:S, :].T
    out += b_out
    return out



# revision 7
# speedup vs baseline: 1.0153x; 1.0153x over previous
"""Bidirectional LSTM (S=2048, B=4096, I=1, H=8, O=1) on 8 Trainium2 NeuronCores.

Strategy (v2)
-------------
Pure data parallel over batch (512 rows/core) plus sequence chunking with
warmup: a chunk started W steps early from zero state converges to the true
trajectory (forget-gate contraction ~0.6/step) before its first emitted
output.

Per core: G=3 pipelined groups x NP=7 chunk-stream pairs (fwd+bwd), chunk
length l=98.  The 7 (fwd,bwd) pairs of a group are stacked block-diagonally:
rhs = [h (112 rows) ; x (14) ; ones (1)] = [127, 512] fp16.

All activations are SIGMOID (one ACT table, maximal merging):
  tanh(x) = 2*sigmoid(2x) - 1
  - g-gate: stationary weights pre-scaled 2x -> psum holds 2*g~;
    g = 2*sig(2g~)-1 folded into DVE scalar_tensor_tensor ops:
       z  = (sig2g - 0.5) * i          [= i*g/2]
       c' = (z * 2) + f*c
  - tanh(c): ACT sigmoid with free scale=2.0; h/2 = (sig2c - 0.5) * o is the
    STORED state, with the 2x folded into the h-columns of all stationaries.

Per group-round (one step of 14 streams):
  PE : 4 matmuls [127x119/112]@[127x512] fp16 -> one psum tile [128,4,512]
       (4 banks); the f-gate stationary carries 7 extra columns computing
       w_out . h for the PREVIOUS step into psum partitions 112..118.
  ACT: ONE merged sigmoid over [112,4,512] (all gates) + sigmoid(2c).
  DVE: f*c (TT) + 3 fused scalar_tensor_tensor ops; h' written straight
       into the next rhs tile (fp16 => 2x DVE mode).
  DMA: next x rows into rhs; out rows [7,512] psum -> HBM (b_out added host
       side).

PSUM = 2 figo tiles x 4 banks = 8 banks, rotating across the 3 groups.
"""

import os
import sys

if "axon" not in os.environ.get("JAX_PLATFORMS", "axon"):
    os.environ["JAX_PLATFORMS"] = "axon,cpu"

try:
    import concourse  # noqa: F401
except ImportError:  # pragma: no cover
    sys.path.insert(0, "/opt/trn_rl_repo")

from contextlib import ExitStack

import numpy as np

import concourse.bacc as bacc
import concourse.mybir as mybir
import concourse.tile as tile

S, B, I, H, O = 2048, 4096, 1, 8, 1
N_CORES = 8
BC = B // N_CORES

NP = 7   # stream pairs per group
G = 3    # pipelined groups per core
W = 12   # warmup rounds per chunk

KH = 16 * NP          # 112 h rows / gate partitions
KR = KH + 2 * NP + 1  # 127 rhs rows (h + x + ones)

GATES = ("f", "i", "g", "o")
TORCH_BLOCK = {"i": 0, "f": 1, "g": 2, "o": 3}

F32 = mybir.dt.float32
F16 = mybir.dt.float16
AF = mybir.ActivationFunctionType
ALU = mybir.AluOpType


def _lchunk():
    return -(-S // (NP * G))  # ceil; tail chunk padded with zero x


# --------------------------------------------------------------------------
# host-side data preparation
# --------------------------------------------------------------------------

def make_weights(wihs, whhs, bihs, bhhs, w_out):
    """Stationary operands [KR, 119|112] fp16.

    Columns 16s+8d..+8 = gate rows of pair s, direction d.  h-block entries
    are 2x (stored h = h/2); the whole g-gate stationary is an extra 2x
    (sigmoid(2*g~) trick).  w_f gets 7 extra columns (112+s) computing
    w_out . h of the step held in rhs.
    """
    out = {}
    for q in GATES:
        bi = TORCH_BLOCK[q]
        ncol = 119 if q == "f" else 112
        sc = 2.0 if q == "g" else 1.0
        w = np.zeros((KR, ncol), np.float32)
        for s in range(NP):
            for d in range(2):
                c0 = 16 * s + 8 * d
                w[c0:c0 + 8, c0:c0 + 8] = 2.0 * sc * whhs[d][8 * bi:8 * bi + 8, :].T
                w[KH + 2 * s + d, c0:c0 + 8] = sc * wihs[d][8 * bi:8 * bi + 8, 0]
                w[KR - 1, c0:c0 + 8] = sc * (bihs[d] + bhhs[d])[8 * bi:8 * bi + 8]
        if q == "f":
            for s in range(NP):
                for d in range(2):
                    c0 = 16 * s + 8 * d
                    w[c0:c0 + 8, 112 + s] = 2.0 * w_out[0, 8 * d:8 * d + 8]
        wm = w.copy()
        wm[:, 0:16] = 0.0  # zero pair-0 gate cols -> chunk-0 state pinned to 0
        out[f"w_{q}"] = w.astype(np.float16)
        out[f"w_{q}_warm"] = wm.astype(np.float16)
    return out


def make_xarr(x_core, future):
    """Per-core x arranged as [G, R+1, 15, bc] fp16; row 14 is ones (bias)."""
    l_chunk = _lchunk()
    R = l_chunk + W
    s_len, bc = x_core.shape
    xb = x_core[(future - np.arange(s_len)) % s_len]
    xa = np.zeros((G, R + 1, 2 * NP + 1, bc), np.float32)
    xa[:, :, 2 * NP, :] = 1.0
    rr = np.arange(R + 1)
    for g in range(G):
        for s in range(NP):
            pos = (g * NP + s) * l_chunk - W + rr
            valid = (pos >= 0) & (pos < s_len)
            for d, src in enumerate((x_core, xb)):
                xa[g, valid, 2 * s + d, :] = src[pos[valid]]
    return xa.astype(np.float16)


def make_in_maps(x, wihs, whhs, bihs, bhhs, w_out, b_out, future):
    shared = make_weights(wihs, whhs, bihs, bhhs, w_out)
    in_maps = []
    for k in range(N_CORES):
        m = dict(shared)
        m["xarr"] = make_xarr(x[:, k * BC:(k + 1) * BC, 0], future)
        in_maps.append(m)
    return in_maps


# --------------------------------------------------------------------------
# program builder
# --------------------------------------------------------------------------

def build_program(num_devices=N_CORES):
    l_chunk = _lchunk()
    R = l_chunk + W
    s_pad = l_chunk * NP * G

    nc = bacc.Bacc("TRN2", target_bir_lowering=False, debug=False,
                   enable_asserts=False, num_devices=num_devices)

    dram = {}
    host_names = []

    def din(name, shape, dt_=F16):
        dram[name] = nc.dram_tensor(name, list(shape), dt_, kind="ExternalInput").ap()
        host_names.append(name)

    for q in GATES:
        ncol = 119 if q == "f" else 112
        din(f"w_{q}", (KR, ncol))
        din(f"w_{q}_warm", (KR, ncol))
    din("xarr", (G, R + 1, 2 * NP + 1, BC))
    out_d = nc.dram_tensor("out", [s_pad, BC], F32, kind="ExternalOutput").ap()
    out_view = out_d.rearrange("(c l) b -> c l b", l=l_chunk)

    with tile.TileContext(nc) as tc, ExitStack() as ctx:
        consts = ctx.enter_context(tc.tile_pool(name="consts", bufs=1))
        rhp = ctx.enter_context(tc.tile_pool(name="rhp", bufs=6))
        up = ctx.enter_context(tc.tile_pool(name="up", bufs=3))
        cp = ctx.enter_context(tc.tile_pool(name="cp", bufs=6))
        tp = ctx.enter_context(tc.tile_pool(name="tp", bufs=3))
        zp = ctx.enter_context(tc.tile_pool(name="zp", bufs=3))
        kp = ctx.enter_context(tc.tile_pool(name="kp", bufs=3))
        obp = ctx.enter_context(tc.tile_pool(name="obp", bufs=3))
        fpp = ctx.enter_context(tc.tile_pool(name="fpp", bufs=2, space="PSUM"))

        ct = {}
        for name, ap in dram.items():
            if name == "xarr":
                continue
            t_ = consts.tile(list(ap.shape), ap.dtype, name=f"c_{name}", tag=f"c_{name}")
            nc.sync.dma_start(out=t_, in_=ap)
            ct[name] = t_

        rhs_cur, c_prev = [], []
        for g in range(G):
            r0 = rhp.tile([KR, BC], F16, name=f"rhs0_{g}", tag="rhs")
            nc.vector.memset(r0[0:KH, :], 0.0)
            nc.sync.dma_start(out=r0[KH:KR, :], in_=dram["xarr"][g, 0])
            c0 = cp.tile([KH, BC], F16, name=f"c0_{g}", tag="c")
            nc.vector.memset(c0, 0.0)
            rhs_cur.append(r0)
            c_prev.append(c0)

        for r in range(R + 1):
            for g in range(G):
                rhs = rhs_cur[g]
                warm = "_warm" if (g == 0 and r < W) else ""
                figo = fpp.tile([128, 4, BC], F32, name=f"ps_{g}_{r}", tag="figo")
                # f-gate matmul also produces w_out.h(r-1) in partitions 112..118
                nc.tensor.matmul(figo[0:119, 0, :], ct[f"w_f{warm}"], rhs,
                                 start=True, stop=True)
                if r >= W + 1:
                    # engine partition access must be 32-aligned: copy the
                    # [96:120) window, DMA only rows 16..23 (the out rows)
                    ob = obp.tile([24, BC], F32, name=f"ob_{g}_{r}", tag="ob")
                    nc.vector.tensor_copy(ob, figo[96:120, 0, :])
                    nc.sync.dma_start(
                        out=out_view[g * NP:(g + 1) * NP, r - 1 - W, :],
                        in_=ob[16:16 + NP, :])
                if r == R:
                    continue  # final round exists only to flush the last outputs
                nc.tensor.matmul(figo[0:KH, 1, :], ct[f"w_i{warm}"], rhs,
                                 start=True, stop=True)
                nc.tensor.matmul(figo[0:KH, 2, :], ct[f"w_g{warm}"], rhs,
                                 start=True, stop=True)
                nc.tensor.matmul(figo[0:KH, 3, :], ct[f"w_o{warm}"], rhs,
                                 start=True, stop=True)

                u = up.tile([KH, 4, BC], F16, name=f"u_{g}_{r}", tag="u")
                nc.scalar.activation(u, figo[0:KH, :, :], AF.Sigmoid)

                tm = tp.tile([KH, BC], F16, name=f"tm_{g}_{r}", tag="tm")
                nc.gpsimd.tensor_mul(tm, u[:, 0, :], c_prev[g])
                z = zp.tile([KH, BC], F16, name=f"z_{g}_{r}", tag="z")
                nc.vector.scalar_tensor_tensor(
                    z, u[:, 2, :], 0.5, u[:, 1, :], ALU.subtract, ALU.mult)
                cn = cp.tile([KH, BC], F16, name=f"c_{g}_{r}", tag="c")
                nc.vector.scalar_tensor_tensor(
                    cn, z, 2.0, tm, ALU.mult, ALU.add)
                tcn = kp.tile([KH, BC], F16, name=f"tc_{g}_{r}", tag="tc")
                nc.scalar.activation(tcn, cn, AF.Sigmoid, scale=2.0)

                rhs_n = rhp.tile([KR, BC], F16, name=f"rhs_{g}_{r}", tag="rhs")
                nc.sync.dma_start(out=rhs_n[KH:KR, :], in_=dram["xarr"][g, r + 1])
                nc.vector.scalar_tensor_tensor(
                    rhs_n[0:KH, :], tcn, 0.5, u[:, 3, :], ALU.subtract, ALU.mult)
                rhs_cur[g], c_prev[g] = rhs_n, cn

    nc.compile()
    return nc, host_names


# --------------------------------------------------------------------------
# runner
# --------------------------------------------------------------------------

_CACHE = {}


def _get_program():
    key = (NP, G, W, BC, S)
    if key not in _CACHE:
        _CACHE[key] = build_program()
    return _CACHE[key]


def kernel(x, w_ih_f, w_hh_f, b_ih_f, b_hh_f, w_ih_b, w_hh_b, b_ih_b, b_hh_b,
           w_out, b_out, future):
    from concourse import bass_utils

    x = np.asarray(x, np.float32)
    wihs = [np.asarray(w_ih_f, np.float32), np.asarray(w_ih_b, np.float32)]
    whhs = [np.asarray(w_hh_f, np.float32), np.asarray(w_hh_b, np.float32)]
    bihs = [np.asarray(b_ih_f, np.float32), np.asarray(b_ih_b, np.float32)]
    bhhs = [np.asarray(b_hh_f, np.float32), np.asarray(b_hh_b, np.float32)]
    w_out = np.asarray(w_out, np.float32)
    b_out = float(np.asarray(b_out).reshape(-1)[0])
    future = int(future)

    nc, names = _get_program()
    in_maps = make_in_maps(x, wihs, whhs, bihs, bhhs, w_out, b_out, future)
    res = bass_utils.run_bass_kernel_spmd(nc, in_maps, core_ids=list(range(N_CORES)))
    out = np.empty((B, S), np.float32)
    for k in range(N_CORES):
        out[k * BC:(k + 1) * BC, :] = res.results[k]["out"][:S, :].T
    out += b_out
    return out


# revision 12
# speedup vs baseline: 1.3045x; 1.2848x over previous
"""Bidirectional LSTM (S=2048, B=4096, I=1, H=8, O=1) on 8 Trainium2 NeuronCores.

Strategy (v2)
-------------
Pure data parallel over batch (512 rows/core) plus sequence chunking with
warmup: a chunk started W steps early from zero state converges to the true
trajectory (forget-gate contraction ~0.6/step) before its first emitted
output.

Per core: G=3 pipelined groups x NP=7 chunk-stream pairs (fwd+bwd), chunk
length l=98.  The 7 (fwd,bwd) pairs of a group are stacked block-diagonally:
rhs = [h (112 rows) ; x (14) ; ones (1)] = [127, 512] fp16.

All activations are SIGMOID (one ACT table, maximal merging):
  tanh(x) = 2*sigmoid(2x) - 1
  - g-gate: stationary weights pre-scaled 2x -> psum holds 2*g~;
    g = 2*sig(2g~)-1 folded into DVE scalar_tensor_tensor ops:
       z  = (sig2g - 0.5) * i          [= i*g/2]
       c' = (z * 2) + f*c
  - tanh(c): ACT sigmoid with free scale=2.0; h/2 = (sig2c - 0.5) * o is the
    STORED state, with the 2x folded into the h-columns of all stationaries.

Per group-round (one step of 14 streams):
  PE : 4 matmuls [127x119/112]@[127x512] fp16 -> one psum tile [128,4,512]
       (4 banks); the f-gate stationary carries 7 extra columns computing
       w_out . h for the PREVIOUS step into psum partitions 112..118.
  ACT: ONE merged sigmoid over [112,4,512] (all gates) + sigmoid(2c).
  DVE: f*c (TT) + 3 fused scalar_tensor_tensor ops; h' written straight
       into the next rhs tile (fp16 => 2x DVE mode).
  DMA: next x rows into rhs; out rows [7,512] psum -> HBM (b_out added host
       side).

PSUM = 2 figo tiles x 4 banks = 8 banks, rotating across the 3 groups.
"""

import os
import sys

if "axon" not in os.environ.get("JAX_PLATFORMS", "axon"):
    os.environ["JAX_PLATFORMS"] = "axon,cpu"

try:
    import concourse  # noqa: F401
except ImportError:  # pragma: no cover
    sys.path.insert(0, "/opt/trn_rl_repo")

from contextlib import ExitStack

import numpy as np

import concourse.bacc as bacc
import concourse.mybir as mybir
import concourse.tile as tile

S, B, I, H, O = 2048, 4096, 1, 8, 1
N_CORES = 8
BC = B // N_CORES

NP = 7   # stream pairs per group
G = 3    # pipelined groups per core
W = 12   # warmup rounds per chunk

KH = 16 * NP          # 112 h rows / gate partitions
KR = KH + 2 * NP + 1  # 127 rhs rows (h + x + ones)

GATES = ("f", "i", "g", "o")
TORCH_BLOCK = {"i": 0, "f": 1, "g": 2, "o": 3}

F32 = mybir.dt.float32
F16 = mybir.dt.float16
AF = mybir.ActivationFunctionType
ALU = mybir.AluOpType


def _lchunk():
    return -(-S // (NP * G))  # ceil; tail chunk padded with zero x


# --------------------------------------------------------------------------
# host-side data preparation
# --------------------------------------------------------------------------

def make_weights(wihs, whhs, bihs, bhhs, w_out):
    """Stationary operands [KR, 119|112] fp16.

    Columns 16s+8d..+8 = gate rows of pair s, direction d.  h-block entries
    are 2x (stored h = h/2); the whole g-gate stationary is an extra 2x
    (sigmoid(2*g~) trick).  w_f gets 7 extra columns (112+s) computing
    w_out . h of the step held in rhs.
    """
    out = {}
    for q in GATES:
        bi = TORCH_BLOCK[q]
        sc = 2.0 if q == "g" else 1.0
        w = np.zeros((KR, 119), np.float32)
        for s in range(NP):
            for d in range(2):
                c0 = 16 * s + 8 * d
                w[c0:c0 + 8, c0:c0 + 8] = 2.0 * sc * whhs[d][8 * bi:8 * bi + 8, :].T
                w[KH + 2 * s + d, c0:c0 + 8] = sc * wihs[d][8 * bi:8 * bi + 8, 0]
                w[KR - 1, c0:c0 + 8] = sc * (bihs[d] + bhhs[d])[8 * bi:8 * bi + 8]
        if q == "f":
            # out columns: psum = sum(w_out * h_stored) = (w_out.h)/2, so the
            # merged sigmoid emits sig(out/2); host applies 2*logit
            for s in range(NP):
                for d in range(2):
                    c0 = 16 * s + 8 * d
                    w[c0:c0 + 8, 112 + s] = w_out[0, 8 * d:8 * d + 8]
        wm = w.copy()
        wm[:, 0:16] = 0.0  # zero pair-0 gate cols -> chunk-0 state pinned to 0
        out[f"w_{q}"] = w.astype(np.float16)
        out[f"w_{q}_warm"] = wm.astype(np.float16)
    return out


def make_xarr(x_core, future):
    """Per-core x arranged as [G, R+1, 15, bc] fp16; row 14 is ones (bias)."""
    l_chunk = _lchunk()
    R = l_chunk + W
    s_len, bc = x_core.shape
    xb = x_core[(future - np.arange(s_len)) % s_len]
    xa = np.zeros((G, R + 1, 2 * NP + 1, bc), np.float32)
    xa[:, :, 2 * NP, :] = 1.0
    rr = np.arange(R + 1)
    for g in range(G):
        for s in range(NP):
            pos = (g * NP + s) * l_chunk - W + rr
            valid = (pos >= 0) & (pos < s_len)
            for d, src in enumerate((x_core, xb)):
                xa[g, valid, 2 * s + d, :] = src[pos[valid]]
    return xa.astype(np.float16)


def make_in_maps(x, wihs, whhs, bihs, bhhs, w_out, b_out, future):
    shared = make_weights(wihs, whhs, bihs, bhhs, w_out)
    in_maps = []
    for k in range(N_CORES):
        m = dict(shared)
        m["xarr"] = make_xarr(x[:, k * BC:(k + 1) * BC, 0], future)
        in_maps.append(m)
    return in_maps


# --------------------------------------------------------------------------
# program builder
# --------------------------------------------------------------------------

def build_program(num_devices=N_CORES):
    l_chunk = _lchunk()
    R = l_chunk + W
    s_pad = l_chunk * NP * G

    nc = bacc.Bacc("TRN2", target_bir_lowering=False, debug=False,
                   enable_asserts=False, num_devices=num_devices)

    dram = {}
    host_names = []

    def din(name, shape, dt_=F16):
        dram[name] = nc.dram_tensor(name, list(shape), dt_, kind="ExternalInput").ap()
        host_names.append(name)

    for q in GATES:
        din(f"w_{q}", (KR, 119))
        din(f"w_{q}_warm", (KR, 119))
    din("xarr", (G, R + 1, 2 * NP + 1, BC))
    out_d = nc.dram_tensor("out", [s_pad, BC], F16, kind="ExternalOutput").ap()
    out_view = out_d.rearrange("(c l) b -> c l b", l=l_chunk)

    with tile.TileContext(nc) as tc, ExitStack() as ctx:
        consts = ctx.enter_context(tc.tile_pool(name="consts", bufs=1))
        rhp = ctx.enter_context(tc.tile_pool(name="rhp", bufs=6))
        up = ctx.enter_context(tc.tile_pool(name="up", bufs=3))
        cp = ctx.enter_context(tc.tile_pool(name="cp", bufs=6))
        tp = ctx.enter_context(tc.tile_pool(name="tp", bufs=3))
        zp = ctx.enter_context(tc.tile_pool(name="zp", bufs=3))
        kp = ctx.enter_context(tc.tile_pool(name="kp", bufs=3))
        fpp = ctx.enter_context(tc.tile_pool(name="fpp", bufs=2, space="PSUM"))

        ct = {}
        for name, ap in dram.items():
            if name == "xarr":
                continue
            t_ = consts.tile(list(ap.shape), ap.dtype, name=f"c_{name}", tag=f"c_{name}")
            nc.sync.dma_start(out=t_, in_=ap)
            ct[name] = t_

        rhs_cur, c_prev = [], []
        for g in range(G):
            r0 = rhp.tile([KR, BC], F16, name=f"rhs0_{g}", tag="rhs")
            nc.vector.memset(r0[0:KH, :], 0.0)
            nc.sync.dma_start(out=r0[KH:KR, :], in_=dram["xarr"][g, 0])
            c0 = cp.tile([KH, BC], F16, name=f"c0_{g}", tag="c")
            nc.vector.memset(c0, 0.0)
            rhs_cur.append(r0)
            c_prev.append(c0)

        for r in range(R + 1):
            for g in range(G):
                rhs = rhs_cur[g]
                warm = "_warm" if (g == 0 and r < W) else ""
                figo = fpp.tile([128, 4, BC], F32, name=f"ps_{g}_{r}", tag="figo")
                # f-gate matmul also produces (w_out.h(r-1))/2 in partitions
                # 112..118; the merged sigmoid turns it into sig(out/2) which
                # the host inverts with 2*logit
                nc.tensor.matmul(figo[0:119, 0, :], ct[f"w_f{warm}"], rhs,
                                 start=True, stop=True)
                if r == R:
                    uf = up.tile([119, 4, BC], F16, name=f"u_{g}_{r}", tag="u")
                    nc.scalar.activation(uf[:, 0, :], figo[0:119, 0, :], AF.Sigmoid)
                    nc.sync.dma_start(
                        out=out_view[g * NP:(g + 1) * NP, r - 1 - W, :],
                        in_=uf[112:119, 0, :])
                    continue  # final round exists only to flush the last outputs
                nc.tensor.matmul(figo[0:119, 1, :], ct[f"w_i{warm}"], rhs,
                                 start=True, stop=True)
                nc.tensor.matmul(figo[0:119, 2, :], ct[f"w_g{warm}"], rhs,
                                 start=True, stop=True)
                nc.tensor.matmul(figo[0:119, 3, :], ct[f"w_o{warm}"], rhs,
                                 start=True, stop=True)

                u = up.tile([119, 4, BC], F16, name=f"u_{g}_{r}", tag="u")
                nc.scalar.activation(u, figo[0:119, :, :], AF.Sigmoid)
                if r >= W + 1:
                    nc.sync.dma_start(
                        out=out_view[g * NP:(g + 1) * NP, r - 1 - W, :],
                        in_=u[112:119, 0, :])

                tm = tp.tile([KH, BC], F16, name=f"tm_{g}_{r}", tag="tm")
                nc.vector.tensor_mul(tm, u[0:KH, 0, :], c_prev[g])
                z = zp.tile([KH, BC], F16, name=f"z_{g}_{r}", tag="z")
                nc.vector.scalar_tensor_tensor(
                    z, u[0:KH, 2, :], 0.5, u[0:KH, 1, :], ALU.subtract, ALU.mult)
                cn = cp.tile([KH, BC], F16, name=f"c_{g}_{r}", tag="c")
                nc.vector.scalar_tensor_tensor(
                    cn, z, 2.0, tm, ALU.mult, ALU.add)
                tcn = kp.tile([KH, BC], F16, name=f"tc_{g}_{r}", tag="tc")
                nc.scalar.activation(tcn, cn, AF.Sigmoid, scale=2.0)

                rhs_n = rhp.tile([KR, BC], F16, name=f"rhs_{g}_{r}", tag="rhs")
                nc.sync.dma_start(out=rhs_n[KH:KR, :], in_=dram["xarr"][g, r + 1])
                nc.vector.scalar_tensor_tensor(
                    rhs_n[0:KH, :], tcn, 0.5, u[0:KH, 3, :], ALU.subtract, ALU.mult)
                rhs_cur[g], c_prev[g] = rhs_n, cn

    nc.compile()
    return nc, host_names


# --------------------------------------------------------------------------
# runner
# --------------------------------------------------------------------------

_CACHE = {}


def _get_program():
    key = (NP, G, W, BC, S)
    if key not in _CACHE:
        _CACHE[key] = build_program()
    return _CACHE[key]


def kernel(x, w_ih_f, w_hh_f, b_ih_f, b_hh_f, w_ih_b, w_hh_b, b_ih_b, b_hh_b,
           w_out, b_out, future):
    from concourse import bass_utils

    x = np.asarray(x, np.float32)
    wihs = [np.asarray(w_ih_f, np.float32), np.asarray(w_ih_b, np.float32)]
    whhs = [np.asarray(w_hh_f, np.float32), np.asarray(w_hh_b, np.float32)]
    bihs = [np.asarray(b_ih_f, np.float32), np.asarray(b_ih_b, np.float32)]
    bhhs = [np.asarray(b_hh_f, np.float32), np.asarray(b_hh_b, np.float32)]
    w_out = np.asarray(w_out, np.float32)
    b_out = float(np.asarray(b_out).reshape(-1)[0])
    future = int(future)

    nc, names = _get_program()
    in_maps = make_in_maps(x, wihs, whhs, bihs, bhhs, w_out, b_out, future)
    res = bass_utils.run_bass_kernel_spmd(nc, in_maps, core_ids=list(range(N_CORES)))
    out = np.empty((B, S), np.float32)
    for k in range(N_CORES):
        u = np.asarray(res.results[k]["out"][:S, :], np.float32).T
        u = np.clip(u, 1e-4, 1.0 - 1e-4)
        out[k * BC:(k + 1) * BC, :] = 2.0 * np.log(u / (1.0 - u))
    out += b_out
    return out
